# revision 3
# baseline (speedup 1.0000x reference)
"""Trainium2 Bass kernel for nn_MultiHeadAttention (B=2, S=4096, D=512, H=8).

Computes: q/k/v = relu(x@W+b) per head, softmax(q k^T / sqrt(64)) v,
out = relu(concat_heads @ Wo + bo).

Sharding: 8 cores = 2 (batch) x 4 (query-slice).  Each core computes full
K/V projections for its batch (redundant across the 4 q-slice cores) and
attention + output projection for its 1024-row query slice.  No collectives;
the host concatenates the 8 output slices.

v2 structure (vs the v1 baseline): attention is organized in "groups" of
(head, ktile-pair): the two ktiles' scores go into one PSUM tile, one exp op
produces fp8 probabilities pT[128, 2, 512], and a single fp8 DoubleRow
matmul (contracting 256 sequence positions) accumulates U.  V is stored in
fp8 with a ones column so U row 64 is the softmax denominator.  exp work is
split between the Scalar engine (exact exp, fp8 out) and the Vector engine
(Schraudolph bit-trick: y=round(s*A+B) as int8, bits reinterpreted as fp8 —
~2.7% rms which the softmax normalization tolerates) so the two engines run
concurrently.  Denominator reciprocals run on ACT as exp(-ln(d)) over the
whole [1, 1024] row; normalization multiplies run on gpsimd.
"""

import numpy as np
import ml_dtypes

import concourse.bass as bass
import concourse.mybir as mybir
import concourse.tile as tile
from concourse import bacc
from concourse import bass_utils

F32 = mybir.dt.float32
BF16 = mybir.dt.bfloat16
FP8 = mybir.dt.float8e4
I8 = mybir.dt.int8
AF = mybir.ActivationFunctionType
ALU = mybir.AluOpType
DR = mybir.MatmulPerfMode.DoubleRow

P = 128
D = 512
H = 8
DH = 64
DT = D // P  # 4 (also = number of head pairs)
B = 2
S = 4096
NCORES = 8
QSPLIT = 4
SQ_FULL = S // QSPLIT  # 1024 query rows per core
QC = 512               # q-chunk (matmul free dim / PSUM bank width)
VP = 80                # padded V row stride (65 used; 80 keeps fp8 16B align)

# exp folding: pT = exp(s/8 + EXPB); the e^EXPB factor cancels in normalize.
EXPB = -2.9
LOG2E = 1.4426950408889634
# DVE bit-trick: int8(round(s*A8 + B8)) bits == fp8e4(exp(s/8 + EXPB))
A8 = (1 << 3) * LOG2E / 8.0
C8 = 0.35
B8 = 7 * (1 << 3) + (1 << 3) * LOG2E * EXPB - C8

# exp engine schedule: cycle over this pattern per group ('a'=ACT, 'd'=DVE)
EXP_PATTERN = "aadad"


def build_mha(sk=S, sq=SQ_FULL, skip_vbias=False):
    """Build the SPMD Bass program (identical on all cores).

    All inputs arrive pre-tiled by the host into exact SBUF layout
    ([128 partitions, contiguous free bytes]) so every load is a max-packet
    linear DMA."""
    nc = bacc.Bacc("TRN2", target_bir_lowering=False, debug=False,
                   num_devices=NCORES)

    xT_d = nc.dram_tensor("xT_bf", (P, DT * sk), BF16,
                          kind="ExternalInput").ap()  # chunk-major, see prep
    xqT_d = nc.dram_tensor("xqT_bf", (P, DT * sq), BF16,
                           kind="ExternalInput").ap()
    w_dram = {}
    for n in ("wq", "wk", "wv", "wo"):
        w_dram[n] = nc.dram_tensor(n, (P, DT * D), BF16,
                                   kind="ExternalInput").ap()
    b_dram = {
        "bq": nc.dram_tensor("bq", (P, DT), F32, kind="ExternalInput").ap(),
        "bk": nc.dram_tensor("bk", (P, DT), F32, kind="ExternalInput").ap(),
        "bv": nc.dram_tensor("bv", (1, D), BF16, kind="ExternalInput").ap(),
        "bo": nc.dram_tensor("bo", (1, D), BF16, kind="ExternalInput").ap(),
    }
    out = nc.dram_tensor("out", (sq, D), F32, kind="ExternalOutput").ap()

    with tile.TileContext(nc) as tc:
        _build_tile(tc, xT_d, xqT_d, w_dram, b_dram, out, sk, sq,
                    skip_vbias)

    nc.compile()
    return nc


def _build_tile(tc, xT_d, xqT_d, w_dram, b_dram, out, sk, sq,
                skip_vbias=False):
    nc = tc.nc
    SK_T = sk // P            # ktiles of the key/value sequence (32)
    NKTP = SK_T // 2          # ktile pairs per head (16)
    SQ_T = sq // P
    NQC = sq // QC            # q chunks per core (2)
    CH = min(4, SK_T)         # stiles per projection chunk
    NCH = SK_T // CH

    exp_sched = {"i": 0}

    def next_exp_engine():
        e = EXP_PATTERN[exp_sched["i"] % len(EXP_PATTERN)]
        exp_sched["i"] += 1
        return e

    with (
        tc.tile_pool(name="singles", bufs=1) as singles,
        tc.tile_pool(name="work", bufs=3) as work,
        tc.tile_pool(name="psum", bufs=2, space="PSUM") as psum,
    ):
        # ---- startup: only what Q-proj pair 0 needs, first ----
        w_bf = {}
        w_bf["wq"] = singles.tile([P, DT, D], BF16, name="wq_bf")
        nc.sync.dma_start(w_bf["wq"], w_dram["wq"].rearrange(
            "p (t n) -> p t n", t=DT))
        b_col = {}
        b_col["bq"] = singles.tile([P, DT], F32, name="bq_col")
        nc.sync.dma_start(b_col["bq"], b_dram["bq"])
        xTq = singles.tile([P, DT, sq], BF16)
        nc.sync.dma_start(xTq, xqT_d.rearrange("p (t s) -> p t s", t=DT))

        QT = singles.tile([P, DT, sq], BF16)

        def qproj(j, nq):
            psQ = psum.tile([P, QC], F32, tag="proj", name="psQ")
            for kt in range(DT):
                nc.tensor.matmul(
                    psQ, w_bf["wq"][:, kt, j * P:(j + 1) * P],
                    xTq[:, kt, nq * QC:(nq + 1) * QC],
                    start=(kt == 0), stop=(kt == DT - 1))
            nc.vector.tensor_scalar(
                QT[:, j, nq * QC:(nq + 1) * QC], psQ,
                b_col["bq"][:, j:j + 1], 0.0, op0=ALU.add, op1=ALU.max)

        qproj(0, 0)
        if NQC > 1:
            qproj(0, 1)

        # ---- K-proj deps next (attention can start before V exists) ----
        b_row = {}
        w_bf["wk"] = singles.tile([P, DT, D], BF16, name="wk_bf")
        nc.sync.dma_start(w_bf["wk"], w_dram["wk"].rearrange(
            "p (t n) -> p t n", t=DT))
        b_col["bk"] = singles.tile([P, DT], F32, name="bk_col")
        nc.sync.dma_start(b_col["bk"], b_dram["bk"])
        CHP = CH * P
        xT = singles.tile([P, NCH, DT, CHP], BF16)
        xT_src = xT_d.rearrange("p (n t s) -> p n t s", n=NCH, t=DT)
        nc.sync.dma_start(xT[:, 0], xT_src[:, 0])
        for n in ("wv", "wo"):
            wb = singles.tile([P, DT, D], BF16, name=f"{n}_bf")
            nc.sync.dma_start(wb, w_dram[n].rearrange(
                "p (t n) -> p t n", t=DT))
            w_bf[n] = wb
            if n == "wv" and not skip_vbias:
                br = singles.tile([1, D], BF16, name="bv_row")
                nc.sync.dma_start(br, b_dram["bv"])
                b_row["bv"] = br
        br = singles.tile([1, D], BF16, name="bo_row")
        nc.sync.dma_start(br, b_dram["bo"])
        b_row["bo"] = br

        # ---- persistent SBUF tensors ----
        bias_t = singles.tile([P, 1], F32)
        nc.vector.memset(bias_t, EXPB)
        xT1 = None
        if not skip_vbias:
            xT1 = singles.tile([1, sk], BF16)
            nc.vector.memset(xT1, 1.0)
        KT = singles.tile([P, DT, sk], BF16)
        V_pad = singles.tile([P, NKTP, H, 2, VP], FP8)
        nc.vector.memset(V_pad[:, :, :, :, DH:DH + 1], 1.0)
        OT = singles.tile([P, DT, sq], BF16)
        OT1 = singles.tile([1, sq], BF16)
        nc.vector.memset(OT1, 1.0)

        # PSUM tags: "proj" 2x1 banks, "scores" 2x2 banks, "psU" 1x2 = 8
        def vproj(st):
            n, si = st // CH, st % CH
            psV = psum.tile([P, D], F32, tag="proj", name="psV")
            for kt in range(DT):
                nc.tensor.matmul(
                    psV, xT[:, n, kt, si * P:(si + 1) * P],
                    w_bf["wv"][:, kt, :],
                    start=(kt == 0),
                    stop=(skip_vbias and kt == DT - 1))
            if not skip_vbias:
                nc.tensor.matmul(psV, xT1[:, st * P:(st + 1) * P],
                                 b_row["bv"], start=False, stop=True)
            nc.vector.tensor_scalar_max(
                V_pad[:, st // 2, :, st % 2, 0:DH],
                psV.rearrange("p (h d) -> p h d", h=H), 0.0)

        def kproj(j, n):
            psK = psum.tile([P, CH * P], F32, tag="proj", name="psK")
            for kt in range(DT):
                nc.tensor.matmul(
                    psK, w_bf["wk"][:, kt, j * P:(j + 1) * P],
                    xT[:, n, kt, :],
                    start=(kt == 0), stop=(kt == DT - 1))
            nc.vector.tensor_scalar(
                KT[:, j, n * CH * P:(n + 1) * CH * P], psK,
                b_col["bk"][:, j:j + 1], 0.0, op0=ALU.add, op1=ALU.max)

        def qk_exp(j, qc, a, t, pt_tag="pT", pt_bufs=5):
            """Scores for one (head, ktile-pair) group -> one exp op -> fp8
            pT[128, 2, QC]."""
            q0 = qc * QC
            h0 = a * DH
            psS = psum.tile([P, 2, QC], F32, tag="scores", bufs=2,
                            name="psS")
            for ko in (0, 1):
                kt = 2 * t + ko
                nc.tensor.matmul(
                    psS[:, ko, :],
                    KT[h0:h0 + DH, j, kt * P:(kt + 1) * P],
                    QT[h0:h0 + DH, j, q0:q0 + QC], start=True, stop=True)
            pT = work.tile([P, 2, QC], FP8, tag=pt_tag, bufs=pt_bufs,
                           name="pT")
            pT_f = pT.rearrange("p a b -> p (a b)")
            psS_f = psS.rearrange("p a b -> p (a b)")
            if next_exp_engine() == "a":
                nc.scalar.activation(pT_f, psS_f, AF.Exp, scale=0.125,
                                     bias=bias_t)
            else:
                nc.vector.tensor_scalar(pT_f.bitcast(I8), psS_f, A8, B8,
                                        op0=ALU.mult, op1=ALU.add)
            return pT

        def u_mm(j, a, t, pT, psU):
            h = 2 * j + a
            nc.tensor.matmul(psU[:, a, :], V_pad[:, t, h, :, 0:DH + 1], pT,
                             start=(t == 0), stop=(t == NKTP - 1),
                             perf_mode=DR)

        def attn_group(j, qc, a, t, psU):
            pT = qk_exp(j, qc, a, t)
            u_mm(j, a, t, pT, psU)

        brc_sink = {}

        def finish_block(j, qc, psU):
            """U done for both heads: copy U rows out of PSUM, compute
            1/denominator on ACT (exp(-ln d)), then normalize on gpsimd."""
            q0 = qc * QC
            ucs = work.tile([DH, 2, QC], F32, tag="ucopy", bufs=2,
                            name="ucs")
            nc.vector.tensor_copy(ucs, psU[0:DH])
            # Ln reads the denominator row at partition 64 and lands it at
            # partition 0 (ACT maps partitions relative to the AP base)
            lnd = work.tile([1, 2 * QC], F32, tag="lnd", bufs=2, name="lnd")
            nc.scalar.activation(
                lnd, psU[DH:DH + 1].rearrange("p a b -> p (a b)"), AF.Ln)
            rcp = work.tile([1, 2 * QC], F32, tag="rcp", bufs=2, name="rcp")
            nc.scalar.activation(rcp, lnd, AF.Exp, scale=-1.0)
            for a in (0, 1):
                h0 = a * DH
                brc = work.tile([DH, QC], F32, tag="brc", bufs=4,
                                name="brc")
                nc.gpsimd.partition_broadcast(
                    brc, rcp[0:1, a * QC:a * QC + QC])
                if a == 0:
                    # all partition bases align (0) -> safe on gpsimd
                    nc.gpsimd.tensor_tensor(
                        OT[0:DH, j, q0:q0 + QC], ucs[:, 0, :], brc,
                        op=ALU.mult)
                else:
                    # dest base 64: DVE handles relative partition bases
                    nc.vector.tensor_mul(
                        OT[h0:h0 + DH, j, q0:q0 + QC], ucs[:, a, :], brc)
                brc_sink[(j, qc)] = brc

        def attn_span(j, qc, ts, psU, fillers=()):
            """Emit groups (2 heads x each ktp in ts), sprinkling fillers
            (deferred work thunks) between groups."""
            fillers = list(fillers)
            ngroups = 2 * len(ts)
            spacing = max(1, ngroups // (len(fillers) + 1))
            gi = 0
            for t in ts:
                for a in (0, 1):
                    attn_group(j, qc, a, t, psU)
                    gi += 1
                    if fillers and gi % spacing == 0:
                        fillers.pop(0)()
            for f in fillers:
                f()
            if ts[-1] == NKTP - 1:
                finish_block(j, qc, psU)

        def new_psU():
            return psum.tile([DH + 1, 2, QC], F32, tag="psU", bufs=1,
                             name="psU")

        def outproj(qt):
            # bias matmul first: it reads OT1, whose re-write after the last
            # normalize acts as a scheduling gate for the whole chain
            psO = psum.tile([P, D], F32, tag="proj", name="psO")
            nc.tensor.matmul(psO, OT1[:, qt * P:(qt + 1) * P],
                             b_row["bo"], start=True, stop=False)
            for j in range(DT):
                nc.tensor.matmul(psO, OT[:, j, qt * P:(qt + 1) * P],
                                 w_bf["wo"][:, j, :],
                                 start=False, stop=(j == DT - 1))
            o_sb = work.tile([P, D], F32, tag="osb", bufs=2, name="o_sb")
            nc.scalar.activation(o_sb, psO, AF.Relu)
            nc.sync.dma_start(out[qt * P:(qt + 1) * P, :], o_sb)

        def gate_outproj(blk):
            """No-op rewrite of OT1 (max(1, rcp<1) == 1) that depends on
            block `blk`'s normalize chain — gates the outproj chains (which
            start with an OT1-reading bias matmul) behind it."""
            brc = brc_sink[blk]
            nc.vector.tensor_scalar(OT1, OT1, brc[0:1, 0:1], None,
                                    op0=ALU.max)

        # ---- chunk loop: x load + V proj + K proj(pair 0) + attn(0, 0) ----
        psU0 = new_psU()
        for n in range(NCH):
            if n > 0:
                nc.sync.dma_start(xT[:, n], xT_src[:, n])
            kproj(0, n)
            tps = [2 * n * CH // 4 + i for i in range(CH // 2)]  # ktps here
            # QK + exp first: exp engines can start before V exists
            pTs = [(a, t, qk_exp(0, 0, a, t)) for t in tps for a in (0, 1)]
            for st in range(n * CH, (n + 1) * CH):
                vproj(st)
            for a, t, pT in pTs:
                u_mm(0, a, t, pT, psU0)
            if tps[-1] == NKTP - 1:
                finish_block(0, 0, psU0)

        # ---- remaining blocks, qc-major; fillers carry the next block's
        # projections plus the first-half output projections ----
        blocks = [(j, 0) for j in range(1, DT)]
        blocks += [(j, 1) for j in range(DT)] if NQC > 1 else []
        owed = {blk: [] for blk in blocks}
        for (j, qc) in blocks:
            if not (j == 0 and qc <= 1):
                owed[(j, qc)].append(lambda j=j, qc=qc: qproj(j, qc))
            if qc == 0 and j >= 1:
                for n in range(NCH):
                    owed[(j, qc)].append(lambda j=j, n=n: kproj(j, n))
        # first-half outproj: OT rows for qc=0 complete after block (DT-1, 0);
        # run them inside the following blocks
        if NQC > 1:
            mid_i = blocks.index((0, 1))
            later = blocks[mid_i + 1]
            owed[later].append(lambda: gate_outproj((DT - 1, 0)))
            for qt in range(SQ_T // NQC):
                owed[later].append(lambda qt=qt: outproj(qt))

        for f in owed[blocks[0]]:
            f()
        for bi, (j, qc) in enumerate(blocks):
            fillers = []
            if bi + 1 < len(blocks):
                fillers += owed[blocks[bi + 1]]
            psU = new_psU()
            attn_span(j, qc, list(range(NKTP)), psU, fillers)

        # ---- tail: last block's normalize + remaining output rows ----
        gate_outproj(blocks[-1])
        qt_lo = SQ_T // NQC if NQC > 1 else 0
        for qt in range(qt_lo, SQ_T):
            outproj(qt)


_NC_CACHE = {}


def _get_nc(sk=S, sq=SQ_FULL, skip_vbias=False):
    key = (sk, sq, skip_vbias)
    if key not in _NC_CACHE:
        _NC_CACHE[key] = build_mha(sk, sq, skip_vbias)
    return _NC_CACHE[key]


def _tile_rows(a):
    """[D, n] -> SBUF layout [P, DT*n]: partition p gets rows p, 128+p, ..."""
    Dd, n = a.shape
    t = Dd // P
    return np.ascontiguousarray(
        a.reshape(t, P, n).transpose(1, 0, 2).reshape(P, t * n))


def _tile_chunks(a, chp):
    """[D, sk] -> chunk-major SBUF layout [P, NCH*DT*chp]: per partition,
    sequence chunks outermost so each chunk is one contiguous linear DMA."""
    Dd, sk = a.shape
    t, nch = Dd // P, sk // chp
    return np.ascontiguousarray(
        a.reshape(t, P, nch, chp).transpose(1, 2, 0, 3).reshape(P, -1))


def prep_inputs(x, Wq, bq, Wk, bk, Wv, bv, Wo, bo):
    """Host-side sharding/layout prep: bf16 casts, feature-major transpose,
    SBUF pre-tiling.  Returns the 8 per-core input maps."""
    bf = ml_dtypes.bfloat16
    x = np.asarray(x, dtype=np.float32)
    shared = {
        "wq": _tile_rows(np.asarray(Wq, np.float32).astype(bf)),
        "wk": _tile_rows(np.asarray(Wk, np.float32).astype(bf)),
        "wv": _tile_rows(np.asarray(Wv, np.float32).astype(bf)),
        "wo": _tile_rows(np.asarray(Wo, np.float32).astype(bf)),
        "bq": np.ascontiguousarray(
            np.asarray(bq, np.float32).reshape(DT, P).T),
        "bk": np.ascontiguousarray(
            np.asarray(bk, np.float32).reshape(DT, P).T),
        "bv": np.asarray(bv, np.float32).astype(bf).reshape(1, D),
        "bo": np.asarray(bo, np.float32).astype(bf).reshape(1, D),
    }
    xT_b = [x[b].T.astype(bf) for b in range(B)]
    xT_tiled = [_tile_chunks(xb, 4 * P) for xb in xT_b]
    in_maps = []
    for c in range(NCORES):
        b, qo = divmod(c, QSPLIT)
        m = dict(shared)
        m["xT_bf"] = xT_tiled[b]
        m["xqT_bf"] = _tile_rows(
            xT_b[b][:, qo * SQ_FULL:(qo + 1) * SQ_FULL])
        in_maps.append(m)
    return in_maps


def kernel(x, Wq, bq, Wk, bk, Wv, bv, Wo, bo, **run_kwargs):
    """Full-input entry point: shards across 8 NeuronCores, returns full out."""
    in_maps = prep_inputs(x, Wq, bq, Wk, bk, Wv, bv, Wo, bo)
    nc = _get_nc(skip_vbias=bool(np.all(np.asarray(bv) == 0)))
    res = bass_utils.run_bass_kernel_spmd(
        nc, in_maps, core_ids=list(range(NCORES)), **run_kwargs)
    full = np.empty((B, S, D), np.float32)
    for c in range(NCORES):
        b, qo = divmod(c, QSPLIT)
        full[b, qo * SQ_FULL:(qo + 1) * SQ_FULL] = res.results[c]["out"]
    if run_kwargs:
        return full, res
    return full


# revision 9
# speedup vs baseline: 1.0430x; 1.0430x over previous
"""Trainium2 Bass kernel for nn_MultiHeadAttention (B=2, S=4096, D=512, H=8).

Computes: q/k/v = relu(x@W+b) per head, softmax(q k^T / sqrt(64)) v,
out = relu(concat_heads @ Wo + bo).

Sharding: 8 cores = 2 (batch) x 4 (query-slice).  Each core computes full
K/V projections for its batch (redundant across the 4 q-slice cores) and
attention + output projection for its 1024-row query slice.  No collectives;
the host concatenates the 8 output slices.

v2 structure (vs the v1 baseline): attention is organized in "groups" of
(head, ktile-pair): the two ktiles' scores go into one PSUM tile, one exp op
produces fp8 probabilities pT[128, 2, 512], and a single fp8 DoubleRow
matmul (contracting 256 sequence positions) accumulates U.  V is stored in
fp8 with a ones column so U row 64 is the softmax denominator.  exp work is
split between the Scalar engine (exact exp, fp8 out) and the Vector engine
(Schraudolph bit-trick: y=round(s*A+B) as int8, bits reinterpreted as fp8 —
~2.7% rms which the softmax normalization tolerates) so the two engines run
concurrently.  Denominator reciprocals run on ACT as exp(-ln(d)) over the
whole [1, 1024] row; normalization multiplies run on gpsimd.
"""

import numpy as np
import ml_dtypes

import concourse.bass as bass
import concourse.mybir as mybir
import concourse.tile as tile
from concourse import bacc
from concourse import bass_utils
from concourse import hw_specs


def _patch_act_tables():
    """Make exp/relu/ln all resolve to the one table set that contains all
    three (natural_log_exp_and_others).  The load-insertion pass assigns
    each ACTIVATE the *first* set containing its function, so a kernel
    mixing exp and ln otherwise reloads tables around every ln (~2.7us per
    switch).  Only set *selection* changes; set contents seen by the
    runtime are untouched."""
    if getattr(hw_specs, "_mha_act_patch", False):
        return
    orig = hw_specs.get_activation_tables
    HOME = "natural_log_exp_and_others"
    AF_ = mybir.ActivationFunctionType

    def patched(arch):
        tables = orig(arch)
        if HOME not in tables:
            return tables
        out = {}
        for name, funcs in tables.items():
            if name != HOME:
                funcs = funcs - {AF_.Exp, AF_.Relu, AF_.Ln}
            out[name] = funcs
        return out

    hw_specs.get_activation_tables = patched
    bacc.get_activation_tables = patched
    hw_specs._mha_act_patch = True

F32 = mybir.dt.float32
BF16 = mybir.dt.bfloat16
FP8 = mybir.dt.float8e4
I8 = mybir.dt.int8
AF = mybir.ActivationFunctionType
ALU = mybir.AluOpType
DR = mybir.MatmulPerfMode.DoubleRow

P = 128
D = 512
H = 8
DH = 64
DT = D // P  # 4 (also = number of head pairs)
B = 2
S = 4096
NCORES = 8
QSPLIT = 4
SQ_FULL = S // QSPLIT  # 1024 query rows per core
QC = 512               # q-chunk (matmul free dim / PSUM bank width)
VP = 80                # padded V row stride (65 used; 80 keeps fp8 16B align)

# exp folding: pT = exp(s/8 + EXPB); the e^EXPB factor cancels in normalize.
EXPB = -2.9
LOG2E = 1.4426950408889634
# DVE bit-trick: int8(round(s*A8 + B8)) bits == fp8e4(exp(s/8 + EXPB))
A8 = (1 << 3) * LOG2E / 8.0
C8 = 0.35
B8 = 7 * (1 << 3) + (1 << 3) * LOG2E * EXPB - C8

# exp engine schedule: cycle over this pattern per group ('a'=ACT, 'd'=DVE)
EXP_PATTERN = "aadad"


def build_mha(sk=S, sq=SQ_FULL, skip_vbias=False):
    """Build the SPMD Bass program (identical on all cores).

    All inputs arrive pre-tiled by the host into exact SBUF layout
    ([128 partitions, contiguous free bytes]) so every load is a max-packet
    linear DMA."""
    _patch_act_tables()
    nc = bacc.Bacc("TRN2", target_bir_lowering=False, debug=False,
                   num_devices=NCORES)

    xT_d = nc.dram_tensor("xT_bf", (P, DT * sk), BF16,
                          kind="ExternalInput").ap()  # chunk-major, see prep
    xqT_d = nc.dram_tensor("xqT_bf", (P, DT * sq), BF16,
                           kind="ExternalInput").ap()
    w_dram = {}
    for n in ("wq", "wk", "wv", "wo"):
        w_dram[n] = nc.dram_tensor(n, (P, DT * D), BF16,
                                   kind="ExternalInput").ap()
    b_dram = {
        "bq": nc.dram_tensor("bq", (P, DT), F32, kind="ExternalInput").ap(),
        "bk": nc.dram_tensor("bk", (P, DT), F32, kind="ExternalInput").ap(),
        "bv": nc.dram_tensor("bv", (1, D), BF16, kind="ExternalInput").ap(),
        "bo": nc.dram_tensor("bo", (1, D), BF16, kind="ExternalInput").ap(),
    }
    out = nc.dram_tensor("out", (sq, D), F32, kind="ExternalOutput").ap()

    with tile.TileContext(nc) as tc:
        _build_tile(tc, xT_d, xqT_d, w_dram, b_dram, out, sk, sq,
                    skip_vbias)

    nc.compile()
    return nc


def _build_tile(tc, xT_d, xqT_d, w_dram, b_dram, out, sk, sq,
                skip_vbias=False):
    nc = tc.nc
    SK_T = sk // P            # ktiles of the key/value sequence (32)
    NKTP = SK_T // 2          # ktile pairs per head (16)
    SQ_T = sq // P
    NQC = sq // QC            # q chunks per core (2)
    CH = min(4, SK_T)         # stiles per projection chunk
    NCH = SK_T // CH

    with (
        tc.tile_pool(name="singles", bufs=1) as singles,
        tc.tile_pool(name="work", bufs=3) as work,
        tc.tile_pool(name="psum", bufs=2, space="PSUM") as psum,
    ):
        # ---- startup: only what Q-proj pair 0 needs, first ----
        w_bf = {}
        w_bf["wq"] = singles.tile([P, DT, D], BF16, name="wq_bf")
        nc.sync.dma_start(w_bf["wq"], w_dram["wq"].rearrange(
            "p (t n) -> p t n", t=DT))
        b_col = {}
        b_col["bq"] = singles.tile([P, DT], F32, name="bq_col")
        nc.sync.dma_start(b_col["bq"], b_dram["bq"])
        xTq = singles.tile([P, DT, sq], BF16)
        nc.sync.dma_start(xTq, xqT_d.rearrange("p (t s) -> p t s", t=DT))

        QT = singles.tile([P, DT, sq], BF16)

        def qproj(j, nq):
            psQ = psum.tile([P, QC], F32, tag="proj", name="psQ")
            for kt in range(DT):
                nc.tensor.matmul(
                    psQ, w_bf["wq"][:, kt, j * P:(j + 1) * P],
                    xTq[:, kt, nq * QC:(nq + 1) * QC],
                    start=(kt == 0), stop=(kt == DT - 1))
            nc.vector.tensor_scalar(
                QT[:, j, nq * QC:(nq + 1) * QC], psQ,
                b_col["bq"][:, j:j + 1], 0.0, op0=ALU.add, op1=ALU.max)

        qproj(0, 0)
        if NQC > 1:
            qproj(0, 1)

        # ---- K-proj deps next (attention can start before V exists) ----
        b_row = {}
        w_bf["wk"] = singles.tile([P, DT, D], BF16, name="wk_bf")
        nc.sync.dma_start(w_bf["wk"], w_dram["wk"].rearrange(
            "p (t n) -> p t n", t=DT))
        b_col["bk"] = singles.tile([P, DT], F32, name="bk_col")
        nc.sync.dma_start(b_col["bk"], b_dram["bk"])
        CHP = CH * P
        xT = singles.tile([P, NCH, DT, CHP], BF16)
        xT_src = xT_d.rearrange("p (n t s) -> p n t s", n=NCH, t=DT)
        nc.sync.dma_start(xT[:, 0], xT_src[:, 0])
        for n in ("wv", "wo"):
            wb = singles.tile([P, DT, D], BF16, name=f"{n}_bf")
            nc.sync.dma_start(wb, w_dram[n].rearrange(
                "p (t n) -> p t n", t=DT))
            w_bf[n] = wb
            if n == "wv" and not skip_vbias:
                br = singles.tile([1, D], BF16, name="bv_row")
                nc.sync.dma_start(br, b_dram["bv"])
                b_row["bv"] = br
        br = singles.tile([1, D], BF16, name="bo_row")
        nc.sync.dma_start(br, b_dram["bo"])
        b_row["bo"] = br

        # ---- persistent SBUF tensors ----
        bias_t = singles.tile([P, 1], F32)
        nc.vector.memset(bias_t, EXPB)
        xT1 = None
        if not skip_vbias:
            xT1 = singles.tile([1, sk], BF16)
            nc.vector.memset(xT1, 1.0)
        KT = singles.tile([P, DT, sk], BF16)
        V_pad = singles.tile([P, NKTP, H, 2, VP], FP8)
        nc.vector.memset(V_pad[:, :, :, :, DH:DH + 1], 1.0)
        OT = singles.tile([P, DT, sq], BF16)
        OT1 = singles.tile([1, sq], BF16)
        nc.vector.memset(OT1, 1.0)

        # PSUM tags: "proj" 2x1 banks, "scores" 2x2 banks, "psU" 1x2 = 8
        def vproj(st):
            n, si = st // CH, st % CH
            psV = psum.tile([P, D], F32, tag="proj", name="psV")
            for kt in range(DT):
                nc.tensor.matmul(
                    psV, xT[:, n, kt, si * P:(si + 1) * P],
                    w_bf["wv"][:, kt, :],
                    start=(kt == 0),
                    stop=(skip_vbias and kt == DT - 1))
            if not skip_vbias:
                nc.tensor.matmul(psV, xT1[:, st * P:(st + 1) * P],
                                 b_row["bv"], start=False, stop=True)
            nc.vector.tensor_scalar_max(
                V_pad[:, st // 2, :, st % 2, 0:DH],
                psV.rearrange("p (h d) -> p h d", h=H), 0.0)

        def kproj(j, n):
            psK = psum.tile([P, CH * P], F32, tag="proj", name="psK")
            for kt in range(DT):
                nc.tensor.matmul(
                    psK, w_bf["wk"][:, kt, j * P:(j + 1) * P],
                    xT[:, n, kt, :],
                    start=(kt == 0), stop=(kt == DT - 1))
            nc.vector.tensor_scalar(
                KT[:, j, n * CH * P:(n + 1) * CH * P], psK,
                b_col["bk"][:, j:j + 1], 0.0, op0=ALU.add, op1=ALU.max)

        def qk_pair(j, qc, t, engines=("a", "d"), pt_tag="pT", pt_bufs=5):
            """Scores + exp for BOTH heads of pair j at ktile-pair t.  The
            four QK matmuls are interleaved head-first so consecutive
            matmuls sit in different PE row groups (partitions 0-63 vs
            64-127) and run concurrently.  One exp per head (engines:
            'a'=ACT exact exp->fp8, 'd'=DVE bit-trick->int8-as-fp8) produce
            fp8 pT[128, 2, QC] tiles for the DoubleRow U matmul."""
            q0 = qc * QC
            psS = [psum.tile([P, 2, QC], F32, tag="scores", bufs=2,
                             name=f"psS{a}") for a in (0, 1)]
            for ko in (0, 1):
                kt = 2 * t + ko
                for a in (0, 1):
                    h0 = a * DH
                    nc.tensor.matmul(
                        psS[a][:, ko, :],
                        KT[h0:h0 + DH, j, kt * P:(kt + 1) * P],
                        QT[h0:h0 + DH, j, q0:q0 + QC], start=True,
                        stop=True)
            pTs = []
            for a in (0, 1):
                pT = work.tile([P, 2, QC], FP8, tag=pt_tag, bufs=pt_bufs,
                               name="pT")
                pT_f = pT.rearrange("p a b -> p (a b)")
                psS_f = psS[a].rearrange("p a b -> p (a b)")
                if engines[a] == "a":
                    nc.scalar.activation(pT_f, psS_f, AF.Exp, scale=0.125,
                                         bias=bias_t)
                else:
                    nc.vector.tensor_scalar(pT_f.bitcast(I8), psS_f, A8, B8,
                                            op0=ALU.mult, op1=ALU.add)
                pTs.append(pT)
            return pTs

        def u_mm(j, a, t, pT, psU):
            h = 2 * j + a
            nc.tensor.matmul(psU[:, a, :], V_pad[:, t, h, :, 0:DH + 1], pT,
                             start=(t == 0), stop=(t == NKTP - 1),
                             perf_mode=DR)

        brc_sink = {}

        def finish_block(j, qc, psU):
            """U done for both heads: copy U rows out of PSUM, compute
            1/denominator on ACT (exp(-ln d)), then normalize on gpsimd."""
            q0 = qc * QC
            ucs = work.tile([DH, 2, QC], F32, tag="ucopy", bufs=2,
                            name="ucs")
            nc.vector.tensor_copy(ucs, psU[0:DH])
            # Ln reads the denominator row at partition 64 and lands it at
            # partition 0 (ACT maps partitions relative to the AP base)
            lnd = work.tile([1, 2 * QC], F32, tag="lnd", bufs=2, name="lnd")
            nc.scalar.activation(
                lnd, psU[DH:DH + 1].rearrange("p a b -> p (a b)"), AF.Ln)
            rcp = work.tile([1, 2 * QC], F32, tag="rcp", bufs=2, name="rcp")
            nc.scalar.activation(rcp, lnd, AF.Exp, scale=-1.0)
            for a in (0, 1):
                h0 = a * DH
                brc = work.tile([DH, QC], F32, tag="brc", bufs=4,
                                name="brc")
                nc.gpsimd.partition_broadcast(
                    brc, rcp[0:1, a * QC:a * QC + QC])
                if a == 0:
                    # all partition bases align (0) -> safe on gpsimd
                    nc.gpsimd.tensor_tensor(
                        OT[0:DH, j, q0:q0 + QC], ucs[:, 0, :], brc,
                        op=ALU.mult)
                else:
                    # dest base 64: DVE handles relative partition bases
                    nc.vector.tensor_mul(
                        OT[h0:h0 + DH, j, q0:q0 + QC], ucs[:, a, :], brc)
                brc_sink[(j, qc)] = brc

        def attn_span(j, qc, ts, psU, fillers=(), pend=None):
            """Emit the ktile-pair iterations of one attention block with a
            one-iteration software pipeline: iteration t emits its QK
            matmuls and exps, then the U matmuls of iteration t-1 (whose
            pT is ready by now), so the in-order PE never waits a full exp
            latency.  Fillers (deferred projections) slot in between."""
            fillers = list(fillers)
            spacing = max(1, len(ts) // (len(fillers) + 1))
            for ti, t in enumerate(ts):
                pTs = qk_pair(j, qc, t)
                if pend is not None:
                    pt0, pTs0 = pend
                    for a in (0, 1):
                        u_mm(j, a, pt0, pTs0[a], psU)
                pend = (t, pTs)
                if fillers and (ti + 1) % spacing == 0:
                    fillers.pop(0)()
            for f in fillers:
                f()
            if ts[-1] == NKTP - 1:
                pt0, pTs0 = pend
                for a in (0, 1):
                    u_mm(j, a, pt0, pTs0[a], psU)
                finish_block(j, qc, psU)
                pend = None
            return pend

        def new_psU():
            return psum.tile([DH + 1, 2, QC], F32, tag="psU", bufs=1,
                             name="psU")

        def outproj(qt):
            # bias matmul first: it reads OT1, whose re-write after the last
            # normalize acts as a scheduling gate for the whole chain
            psO = psum.tile([P, D], F32, tag="proj", name="psO")
            nc.tensor.matmul(psO, OT1[:, qt * P:(qt + 1) * P],
                             b_row["bo"], start=True, stop=False)
            for j in range(DT):
                nc.tensor.matmul(psO, OT[:, j, qt * P:(qt + 1) * P],
                                 w_bf["wo"][:, j, :],
                                 start=False, stop=(j == DT - 1))
            o_sb = work.tile([P, D], F32, tag="osb", bufs=2, name="o_sb")
            nc.scalar.activation(o_sb, psO, AF.Relu)
            nc.sync.dma_start(out[qt * P:(qt + 1) * P, :], o_sb)

        def gate_outproj(blk):
            """No-op rewrite of OT1 (max(1, rcp<1) == 1) that depends on
            block `blk`'s normalize chain — gates the outproj chains (which
            start with an OT1-reading bias matmul) behind it."""
            brc = brc_sink[blk]
            nc.vector.tensor_scalar(OT1, OT1, brc[0:1, 0:1], None,
                                    op0=ALU.max)

        # ---- chunk loop: x load + V proj + K proj(pair 0) + attn(0, 0);
        # exps overlap the vproj matmuls, U runs after its V is written ----
        psU0 = new_psU()
        for n in range(NCH):
            if n > 0:
                nc.sync.dma_start(xT[:, n], xT_src[:, n])
            kproj(0, n)
            t0, t1 = 2 * n, 2 * n + 1
            pTs0 = qk_pair(0, 0, t0)
            vproj(2 * t0)
            vproj(2 * t0 + 1)
            pTs1 = qk_pair(0, 0, t1)
            vproj(2 * t1)
            vproj(2 * t1 + 1)
            for t, pTs in ((t0, pTs0), (t1, pTs1)):
                for a in (0, 1):
                    u_mm(0, a, t, pTs[a], psU0)
            if t1 == NKTP - 1:
                finish_block(0, 0, psU0)

        # ---- remaining blocks, qc-major; fillers carry the next block's
        # projections plus the first-half output projections ----
        blocks = [(j, 0) for j in range(1, DT)]
        blocks += [(j, 1) for j in range(DT)] if NQC > 1 else []
        owed = {blk: [] for blk in blocks}
        for (j, qc) in blocks:
            if not (j == 0 and qc <= 1):
                owed[(j, qc)].append(lambda j=j, qc=qc: qproj(j, qc))
            if qc == 0 and j >= 1:
                for n in range(NCH):
                    owed[(j, qc)].append(lambda j=j, n=n: kproj(j, n))
        # first-half outproj: OT rows for qc=0 complete after block (DT-1, 0);
        # run them inside the following blocks
        if NQC > 1:
            mid_i = blocks.index((0, 1))
            later = blocks[mid_i + 1]
            owed[later].append(lambda: gate_outproj((DT - 1, 0)))
            for qt in range(SQ_T // NQC):
                owed[later].append(lambda qt=qt: outproj(qt))

        for f in owed[blocks[0]]:
            f()
        for bi, (j, qc) in enumerate(blocks):
            fillers = []
            if bi + 1 < len(blocks):
                fillers += owed[blocks[bi + 1]]
            psU = new_psU()
            attn_span(j, qc, list(range(NKTP)), psU, fillers)

        # ---- tail: last block's normalize + remaining output rows ----
        gate_outproj(blocks[-1])
        qt_lo = SQ_T // NQC if NQC > 1 else 0
        for qt in range(qt_lo, SQ_T):
            outproj(qt)


_NC_CACHE = {}


def _get_nc(sk=S, sq=SQ_FULL, skip_vbias=False):
    key = (sk, sq, skip_vbias)
    if key not in _NC_CACHE:
        _NC_CACHE[key] = build_mha(sk, sq, skip_vbias)
    return _NC_CACHE[key]


def _tile_rows(a):
    """[D, n] -> SBUF layout [P, DT*n]: partition p gets rows p, 128+p, ..."""
    Dd, n = a.shape
    t = Dd // P
    return np.ascontiguousarray(
        a.reshape(t, P, n).transpose(1, 0, 2).reshape(P, t * n))


def _tile_chunks(a, chp):
    """[D, sk] -> chunk-major SBUF layout [P, NCH*DT*chp]: per partition,
    sequence chunks outermost so each chunk is one contiguous linear DMA."""
    Dd, sk = a.shape
    t, nch = Dd // P, sk // chp
    return np.ascontiguousarray(
        a.reshape(t, P, nch, chp).transpose(1, 2, 0, 3).reshape(P, -1))


def prep_inputs(x, Wq, bq, Wk, bk, Wv, bv, Wo, bo):
    """Host-side sharding/layout prep: bf16 casts, feature-major transpose,
    SBUF pre-tiling.  Returns the 8 per-core input maps."""
    bf = ml_dtypes.bfloat16
    x = np.asarray(x, dtype=np.float32)
    shared = {
        "wq": _tile_rows(np.asarray(Wq, np.float32).astype(bf)),
        "wk": _tile_rows(np.asarray(Wk, np.float32).astype(bf)),
        "wv": _tile_rows(np.asarray(Wv, np.float32).astype(bf)),
        "wo": _tile_rows(np.asarray(Wo, np.float32).astype(bf)),
        "bq": np.ascontiguousarray(
            np.asarray(bq, np.float32).reshape(DT, P).T),
        "bk": np.ascontiguousarray(
            np.asarray(bk, np.float32).reshape(DT, P).T),
        "bv": np.asarray(bv, np.float32).astype(bf).reshape(1, D),
        "bo": np.asarray(bo, np.float32).astype(bf).reshape(1, D),
    }
    xT_b = [x[b].T.astype(bf) for b in range(B)]
    xT_tiled = [_tile_chunks(xb, 4 * P) for xb in xT_b]
    in_maps = []
    for c in range(NCORES):
        b, qo = divmod(c, QSPLIT)
        m = dict(shared)
        m["xT_bf"] = xT_tiled[b]
        m["xqT_bf"] = _tile_rows(
            xT_b[b][:, qo * SQ_FULL:(qo + 1) * SQ_FULL])
        in_maps.append(m)
    return in_maps


def kernel(x, Wq, bq, Wk, bk, Wv, bv, Wo, bo, **run_kwargs):
    """Full-input entry point: shards across 8 NeuronCores, returns full out."""
    in_maps = prep_inputs(x, Wq, bq, Wk, bk, Wv, bv, Wo, bo)
    nc = _get_nc(skip_vbias=bool(np.all(np.asarray(bv) == 0)))
    res = bass_utils.run_bass_kernel_spmd(
        nc, in_maps, core_ids=list(range(NCORES)), **run_kwargs)
    full = np.empty((B, S, D), np.float32)
    for c in range(NCORES):
        b, qo = divmod(c, QSPLIT)
        full[b, qo * SQ_FULL:(qo + 1) * SQ_FULL] = res.results[c]["out"]
    if run_kwargs:
        return full, res
    return full


# revision 15
# speedup vs baseline: 1.3917x; 1.3343x over previous
"""Trainium2 Bass kernel for nn_MultiHeadAttention (B=2, S=4096, D=512, H=8).

Computes: q/k/v = relu(x@W+b) per head, softmax(q k^T / sqrt(64)) v,
out = relu(concat_heads @ Wo + bo).

Sharding: 8 cores = 2 (batch) x 4 (query-slice).  Each core computes full
K/V projections for its batch (redundant across the 4 q-slice cores) and
attention + output projection for its 1024-row query slice.  No collectives;
the host concatenates the 8 output slices.

v2 structure (vs the v1 baseline): attention is organized in "groups" of
(head, ktile-pair): the two ktiles' scores go into one PSUM tile, one exp op
produces fp8 probabilities pT[128, 2, 512], and a single fp8 DoubleRow
matmul (contracting 256 sequence positions) accumulates U.  V is stored in
fp8 with a ones column so U row 64 is the softmax denominator.  exp work is
split between the Scalar engine (exact exp, fp8 out) and the Vector engine
(Schraudolph bit-trick: y=round(s*A+B) as int8, bits reinterpreted as fp8 —
~2.7% rms which the softmax normalization tolerates) so the two engines run
concurrently.  Denominator reciprocals run on ACT as exp(-ln(d)) over the
whole [1, 1024] row; normalization multiplies run on gpsimd.
"""

import numpy as np
import ml_dtypes

import concourse.bass as bass
import concourse.mybir as mybir
import concourse.tile as tile
from concourse import bacc
from concourse import bass_utils
from concourse import hw_specs


def _patch_act_tables():
    """Make exp/relu/ln all resolve to the one table set that contains all
    three (natural_log_exp_and_others).  The load-insertion pass assigns
    each ACTIVATE the *first* set containing its function, so a kernel
    mixing exp and ln otherwise reloads tables around every ln (~2.7us per
    switch).  Only set *selection* changes; set contents seen by the
    runtime are untouched."""
    if getattr(hw_specs, "_mha_act_patch", False):
        return
    orig = hw_specs.get_activation_tables
    HOME = "natural_log_exp_and_others"
    AF_ = mybir.ActivationFunctionType

    def patched(arch):
        tables = orig(arch)
        if HOME not in tables:
            return tables
        out = {}
        for name, funcs in tables.items():
            if name != HOME:
                funcs = funcs - {AF_.Exp, AF_.Relu, AF_.Ln}
            out[name] = funcs
        return out

    hw_specs.get_activation_tables = patched
    bacc.get_activation_tables = patched
    hw_specs._mha_act_patch = True

F32 = mybir.dt.float32
BF16 = mybir.dt.bfloat16
FP8 = mybir.dt.float8e4
I8 = mybir.dt.int8
AF = mybir.ActivationFunctionType
ALU = mybir.AluOpType
DR = mybir.MatmulPerfMode.DoubleRow

P = 128
D = 512
H = 8
DH = 64
DT = D // P  # 4 (also = number of head pairs)
B = 2
S = 4096
NCORES = 8
QSPLIT = 4
SQ_FULL = S // QSPLIT  # 1024 query rows per core
QC = 512               # q-chunk (matmul free dim / PSUM bank width)
VP = 80                # padded V row stride (65 used; 80 keeps fp8 16B align)

# exp folding: pT = exp(s/8 + EXPB); the e^EXPB factor cancels in normalize.
EXPB = -2.9
LOG2E = 1.4426950408889634
# DVE bit-trick: int8(round(s*A8 + B8)) bits == fp8e4(exp(s/8 + EXPB))
A8 = (1 << 3) * LOG2E / 8.0
C8 = 0.35
B8 = 7 * (1 << 3) + (1 << 3) * LOG2E * EXPB - C8

# exp engine schedule, indexed by ktile ('a'=ACT exact, 'd'=DVE bit-trick)
EXP_PATTERN = "ad"


def build_mha(sk=S, sq=SQ_FULL, skip_vbias=False):
    """Build the SPMD Bass program (identical on all cores).

    All inputs arrive pre-tiled by the host into exact SBUF layout
    ([128 partitions, contiguous free bytes]) so every load is a max-packet
    linear DMA."""
    _patch_act_tables()
    nc = bacc.Bacc("TRN2", target_bir_lowering=False, debug=False,
                   num_devices=NCORES)

    xT_d = nc.dram_tensor("xT_bf", (P, DT * sk), BF16,
                          kind="ExternalInput").ap()  # chunk-major, see prep
    xqT_d = nc.dram_tensor("xqT_bf", (P, DT * sq), BF16,
                           kind="ExternalInput").ap()
    w_dram = {}
    for n in ("wq", "wk", "wv", "wo"):
        w_dram[n] = nc.dram_tensor(n, (P, DT * D), BF16,
                                   kind="ExternalInput").ap()
    b_dram = {
        "bq": nc.dram_tensor("bq", (P, DT), F32, kind="ExternalInput").ap(),
        "bk": nc.dram_tensor("bk", (P, DT), F32, kind="ExternalInput").ap(),
        "bv": nc.dram_tensor("bv", (1, D), BF16, kind="ExternalInput").ap(),
        "bo": nc.dram_tensor("bo", (1, D), BF16, kind="ExternalInput").ap(),
    }
    out = nc.dram_tensor("out", (sq, D), F32, kind="ExternalOutput").ap()

    with tile.TileContext(nc) as tc:
        _build_tile(tc, xT_d, xqT_d, w_dram, b_dram, out, sk, sq,
                    skip_vbias)

    nc.compile()
    return nc


def _build_tile(tc, xT_d, xqT_d, w_dram, b_dram, out, sk, sq,
                skip_vbias=False):
    nc = tc.nc
    SK_T = sk // P            # ktiles of the key/value sequence (32)
    NKTP = SK_T // 2          # ktile pairs per head (16)
    SQ_T = sq // P
    NQC = sq // QC            # q chunks per core (2)
    CH = min(4, SK_T)         # stiles per projection chunk
    NCH = SK_T // CH

    with (
        tc.tile_pool(name="singles", bufs=1) as singles,
        tc.tile_pool(name="work", bufs=3) as work,
        tc.tile_pool(name="psum", bufs=2, space="PSUM") as psum,
    ):
        # ---- startup: only what Q-proj pair 0 needs, first ----
        w_bf = {}
        w_bf["wq"] = singles.tile([P, DT, D], BF16, name="wq_bf")
        nc.sync.dma_start(w_bf["wq"], w_dram["wq"].rearrange(
            "p (t n) -> p t n", t=DT))
        b_col = {}
        b_col["bq"] = singles.tile([P, DT], F32, name="bq_col")
        nc.sync.dma_start(b_col["bq"], b_dram["bq"])
        xTq = singles.tile([P, DT, sq], BF16)
        nc.sync.dma_start(xTq, xqT_d.rearrange("p (t s) -> p t s", t=DT))

        QT = singles.tile([P, DT, sq], BF16)

        def qproj(j, nq):
            psQ = psum.tile([P, QC], F32, tag="proj", name="psQ")
            for kt in range(DT):
                nc.tensor.matmul(
                    psQ, w_bf["wq"][:, kt, j * P:(j + 1) * P],
                    xTq[:, kt, nq * QC:(nq + 1) * QC],
                    start=(kt == 0), stop=(kt == DT - 1))
            nc.vector.tensor_scalar(
                QT[:, j, nq * QC:(nq + 1) * QC], psQ,
                b_col["bq"][:, j:j + 1], 0.0, op0=ALU.add, op1=ALU.max)

        qproj(0, 0)
        if NQC > 1:
            qproj(0, 1)

        # ---- K-proj deps next (attention can start before V exists) ----
        b_row = {}
        w_bf["wk"] = singles.tile([P, DT, D], BF16, name="wk_bf")
        nc.sync.dma_start(w_bf["wk"], w_dram["wk"].rearrange(
            "p (t n) -> p t n", t=DT))
        b_col["bk"] = singles.tile([P, DT], F32, name="bk_col")
        nc.sync.dma_start(b_col["bk"], b_dram["bk"])
        CHP = CH * P
        xT = singles.tile([P, NCH, DT, CHP], BF16)
        xT_src = xT_d.rearrange("p (n t s) -> p n t s", n=NCH, t=DT)
        nc.sync.dma_start(xT[:, 0], xT_src[:, 0])
        for n in ("wv", "wo"):
            wb = singles.tile([P, DT, D], BF16, name=f"{n}_bf")
            nc.sync.dma_start(wb, w_dram[n].rearrange(
                "p (t n) -> p t n", t=DT))
            w_bf[n] = wb
            if n == "wv" and not skip_vbias:
                br = singles.tile([1, D], BF16, name="bv_row")
                nc.sync.dma_start(br, b_dram["bv"])
                b_row["bv"] = br
        br = singles.tile([1, D], BF16, name="bo_row")
        nc.sync.dma_start(br, b_dram["bo"])
        b_row["bo"] = br

        # ---- persistent SBUF tensors ----
        bias_t = singles.tile([P, 1], F32)
        nc.vector.memset(bias_t, EXPB)
        xT1 = None
        if not skip_vbias:
            xT1 = singles.tile([1, sk], BF16)
            nc.vector.memset(xT1, 1.0)
        KT = singles.tile([P, DT, sk], BF16)
        V_pad = singles.tile([P, NKTP, H, 2, VP], FP8)
        nc.vector.memset(V_pad[:, :, :, :, DH:DH + 1], 1.0)
        OT = singles.tile([P, DT, sq], BF16)
        OT1 = singles.tile([1, sq], BF16)
        nc.vector.memset(OT1, 1.0)

        # PSUM tags: "proj" 2x1 banks, "scores" 2x2 banks, "psU" 1x2 = 8
        def vproj(st):
            n, si = st // CH, st % CH
            psV = psum.tile([P, D], F32, tag="proj", name="psV")
            for kt in range(DT):
                nc.tensor.matmul(
                    psV, xT[:, n, kt, si * P:(si + 1) * P],
                    w_bf["wv"][:, kt, :],
                    start=(kt == 0),
                    stop=(skip_vbias and kt == DT - 1))
            if not skip_vbias:
                nc.tensor.matmul(psV, xT1[:, st * P:(st + 1) * P],
                                 b_row["bv"], start=False, stop=True)
            nc.vector.tensor_scalar_max(
                V_pad[:, st // 2, :, st % 2, 0:DH],
                psV.rearrange("p (h d) -> p h d", h=H), 0.0)

        def kproj(j, n):
            psK = psum.tile([P, CH * P], F32, tag="proj", name="psK")
            for kt in range(DT):
                nc.tensor.matmul(
                    psK, w_bf["wk"][:, kt, j * P:(j + 1) * P],
                    xT[:, n, kt, :],
                    start=(kt == 0), stop=(kt == DT - 1))
            nc.vector.tensor_scalar(
                KT[:, j, n * CH * P:(n + 1) * CH * P], psK,
                b_col["bk"][:, j:j + 1], 0.0, op0=ALU.add, op1=ALU.max)

        # fp8 probability ring: slot kt%RING holds exp'd scores for both
        # heads of one ktile; the U matmul reads two adjacent slots with a
        # strided DoubleRow access pattern.
        RING = 8
        PT = singles.tile([P, RING, 2, QC], FP8, name="PT_ring")

        def qk1(j, qc, kt, eng):
            """Scores + exp for BOTH heads of pair j at ktile kt.  The two
            QK matmuls sit in different PE row groups (partitions 0-63 vs
            64-127) and run concurrently; one 1024-wide exp op (eng 'a' =
            ACT exact exp->fp8, 'd' = DVE bit-trick int8-as-fp8) covers
            both heads."""
            q0 = qc * QC
            psS = psum.tile([P, 2, QC], F32, tag="scores", bufs=2,
                            name="psS")
            for a in (0, 1):
                h0 = a * DH
                nc.tensor.matmul(
                    psS[:, a, :],
                    KT[h0:h0 + DH, j, kt * P:(kt + 1) * P],
                    QT[h0:h0 + DH, j, q0:q0 + QC], start=True, stop=True)
            slot = kt % RING
            pT_f = PT[:, slot].rearrange("p a b -> p (a b)")
            psS_f = psS.rearrange("p a b -> p (a b)")
            if eng == "a":
                nc.scalar.activation(pT_f, psS_f, AF.Exp, scale=0.125,
                                     bias=bias_t)
            else:
                nc.vector.tensor_scalar(pT_f.bitcast(I8), psS_f, A8, B8,
                                        op0=ALU.mult, op1=ALU.add)

        def u_pair(j, tp, psU):
            """DoubleRow U matmuls for both heads of ktile pair tp, reading
            ring slots (2tp)%RING, (2tp)%RING+1 (slot stride 2*QC fp8)."""
            s0 = (2 * tp) % RING
            for a in (0, 1):
                nc.tensor.matmul(
                    psU[:, a, :], V_pad[:, tp, 2 * j + a, :, 0:DH + 1],
                    PT[:, s0:s0 + 2, a, :],
                    start=(tp == 0), stop=(tp == NKTP - 1), perf_mode=DR)

        brc_sink = {}

        def finish_block(j, qc, psU):
            """U done for both heads: copy U rows out of PSUM, compute
            1/denominator on ACT (exp(-ln d)), then normalize on gpsimd."""
            q0 = qc * QC
            ucs = work.tile([DH, 2, QC], F32, tag="ucopy", bufs=2,
                            name="ucs")
            nc.vector.tensor_copy(ucs, psU[0:DH])
            # Ln reads the denominator row at partition 64 and lands it at
            # partition 0 (ACT maps partitions relative to the AP base)
            lnd = work.tile([1, 2 * QC], F32, tag="lnd", bufs=2, name="lnd")
            nc.scalar.activation(
                lnd, psU[DH:DH + 1].rearrange("p a b -> p (a b)"), AF.Ln)
            rcp = work.tile([1, 2 * QC], F32, tag="rcp", bufs=2, name="rcp")
            nc.scalar.activation(rcp, lnd, AF.Exp, scale=-1.0)
            for a in (0, 1):
                h0 = a * DH
                brc = work.tile([DH, QC], F32, tag="brc", bufs=4,
                                name="brc")
                nc.gpsimd.partition_broadcast(
                    brc, rcp[0:1, a * QC:a * QC + QC])
                if a == 0:
                    # all partition bases align (0) -> safe on gpsimd
                    nc.gpsimd.tensor_tensor(
                        OT[0:DH, j, q0:q0 + QC], ucs[:, 0, :], brc,
                        op=ALU.mult)
                else:
                    # dest base 64: DVE handles relative partition bases
                    nc.vector.tensor_mul(
                        OT[h0:h0 + DH, j, q0:q0 + QC], ucs[:, a, :], brc)
                brc_sink[(j, qc)] = brc

        def attn_span(j, qc, kts, psU, fillers=()):
            """Emit one attention block: per ktile a QK pair + exp
            (engines alternating by ktile), with the U matmul pair lagging
            two ktiles behind so the in-order PE never waits on an exp.
            Fillers (deferred projections) slot in between ktiles."""
            fillers = list(fillers)
            spacing = max(1, len(kts) // (len(fillers) + 1))
            for i, kt in enumerate(kts):
                eng = EXP_PATTERN[kt % len(EXP_PATTERN)]
                qk1(j, qc, kt, eng)
                if kt >= 3 and kt % 2 == 1:
                    u_pair(j, (kt - 3) // 2, psU)
                if fillers and (i + 1) % spacing == 0:
                    fillers.pop(0)()
            for f in fillers:
                f()
            if kts[-1] == SK_T - 1:
                u_pair(j, NKTP - 1, psU)
                finish_block(j, qc, psU)

        def new_psU():
            return psum.tile([DH + 1, 2, QC], F32, tag="psU", bufs=1,
                             name="psU")

        def outproj(qt):
            # bias matmul first: it reads OT1, whose re-write after the last
            # normalize acts as a scheduling gate for the whole chain
            psO = psum.tile([P, D], F32, tag="proj", name="psO")
            nc.tensor.matmul(psO, OT1[:, qt * P:(qt + 1) * P],
                             b_row["bo"], start=True, stop=False)
            for j in range(DT):
                nc.tensor.matmul(psO, OT[:, j, qt * P:(qt + 1) * P],
                                 w_bf["wo"][:, j, :],
                                 start=False, stop=(j == DT - 1))
            o_sb = work.tile([P, D], F32, tag="osb", bufs=2, name="o_sb")
            nc.scalar.activation(o_sb, psO, AF.Relu)
            nc.sync.dma_start(out[qt * P:(qt + 1) * P, :], o_sb)

        def gate_outproj(blk):
            """No-op rewrite of OT1 (max(1, rcp<1) == 1) that depends on
            block `blk`'s normalize chain — gates the outproj chains (which
            start with an OT1-reading bias matmul) behind it."""
            brc = brc_sink[blk]
            nc.vector.tensor_scalar(OT1, OT1, brc[0:1, 0:1], None,
                                    op0=ALU.max)

        # ---- chunk loop: x load + V proj + K proj(pair 0) + attn(0, 0);
        # exps overlap the vproj matmuls, U runs after its V is written ----
        psU0 = new_psU()
        for n in range(NCH):
            if n > 0:
                nc.sync.dma_start(xT[:, n], xT_src[:, n])
            kproj(0, n)
            for i in range(CH // 2):
                kt0 = n * CH + 2 * i
                qk1(0, 0, kt0, EXP_PATTERN[kt0 % len(EXP_PATTERN)])
                qk1(0, 0, kt0 + 1,
                    EXP_PATTERN[(kt0 + 1) % len(EXP_PATTERN)])
                vproj(kt0)
                vproj(kt0 + 1)
            for i in range(CH // 2):
                u_pair(0, n * CH // 2 + i, psU0)
            if (n + 1) * CH == SK_T:
                finish_block(0, 0, psU0)

        # ---- remaining blocks, qc-major; fillers carry the next block's
        # projections plus the first-half output projections ----
        blocks = [(j, 0) for j in range(1, DT)]
        blocks += [(j, 1) for j in range(DT)] if NQC > 1 else []
        owed = {blk: [] for blk in blocks}
        for (j, qc) in blocks:
            if not (j == 0 and qc <= 1):
                owed[(j, qc)].append(lambda j=j, qc=qc: qproj(j, qc))
            if qc == 0 and j >= 1:
                for n in range(NCH):
                    owed[(j, qc)].append(lambda j=j, n=n: kproj(j, n))
        # first-half outproj: OT rows for qc=0 complete after block (DT-1, 0);
        # run them inside the following blocks
        if NQC > 1:
            mid_i = blocks.index((0, 1))
            later = blocks[mid_i + 1]
            owed[later].append(lambda: gate_outproj((DT - 1, 0)))
            for qt in range(SQ_T // NQC):
                owed[later].append(lambda qt=qt: outproj(qt))

        for f in owed[blocks[0]]:
            f()
        for bi, (j, qc) in enumerate(blocks):
            fillers = []
            if bi + 1 < len(blocks):
                fillers += owed[blocks[bi + 1]]
            psU = new_psU()
            attn_span(j, qc, list(range(SK_T)), psU, fillers)

        # ---- tail: last block's normalize + remaining output rows ----
        gate_outproj(blocks[-1])
        qt_lo = SQ_T // NQC if NQC > 1 else 0
        for qt in range(qt_lo, SQ_T):
            outproj(qt)


_NC_CACHE = {}


def _get_nc(sk=S, sq=SQ_FULL, skip_vbias=False):
    key = (sk, sq, skip_vbias)
    if key not in _NC_CACHE:
        _NC_CACHE[key] = build_mha(sk, sq, skip_vbias)
    return _NC_CACHE[key]


def _tile_rows(a):
    """[D, n] -> SBUF layout [P, DT*n]: partition p gets rows p, 128+p, ..."""
    Dd, n = a.shape
    t = Dd // P
    return np.ascontiguousarray(
        a.reshape(t, P, n).transpose(1, 0, 2).reshape(P, t * n))


def _tile_chunks(a, chp):
    """[D, sk] -> chunk-major SBUF layout [P, NCH*DT*chp]: per partition,
    sequence chunks outermost so each chunk is one contiguous linear DMA."""
    Dd, sk = a.shape
    t, nch = Dd // P, sk // chp
    return np.ascontiguousarray(
        a.reshape(t, P, nch, chp).transpose(1, 2, 0, 3).reshape(P, -1))


def prep_inputs(x, Wq, bq, Wk, bk, Wv, bv, Wo, bo):
    """Host-side sharding/layout prep: bf16 casts, feature-major transpose,
    SBUF pre-tiling.  Returns the 8 per-core input maps."""
    bf = ml_dtypes.bfloat16
    x = np.asarray(x, dtype=np.float32)
    shared = {
        "wq": _tile_rows(np.asarray(Wq, np.float32).astype(bf)),
        "wk": _tile_rows(np.asarray(Wk, np.float32).astype(bf)),
        "wv": _tile_rows(np.asarray(Wv, np.float32).astype(bf)),
        "wo": _tile_rows(np.asarray(Wo, np.float32).astype(bf)),
        "bq": np.ascontiguousarray(
            np.asarray(bq, np.float32).reshape(DT, P).T),
        "bk": np.ascontiguousarray(
            np.asarray(bk, np.float32).reshape(DT, P).T),
        "bv": np.asarray(bv, np.float32).astype(bf).reshape(1, D),
        "bo": np.asarray(bo, np.float32).astype(bf).reshape(1, D),
    }
    xT_b = [x[b].T.astype(bf) for b in range(B)]
    xT_tiled = [_tile_chunks(xb, 4 * P) for xb in xT_b]
    in_maps = []
    for c in range(NCORES):
        b, qo = divmod(c, QSPLIT)
        m = dict(shared)
        m["xT_bf"] = xT_tiled[b]
        m["xqT_bf"] = _tile_rows(
            xT_b[b][:, qo * SQ_FULL:(qo + 1) * SQ_FULL])
        in_maps.append(m)
    return in_maps


def kernel(x, Wq, bq, Wk, bk, Wv, bv, Wo, bo, **run_kwargs):
    """Full-input entry point: shards across 8 NeuronCores, returns full out."""
    in_maps = prep_inputs(x, Wq, bq, Wk, bk, Wv, bv, Wo, bo)
    nc = _get_nc(skip_vbias=bool(np.all(np.asarray(bv) == 0)))
    res = bass_utils.run_bass_kernel_spmd(
        nc, in_maps, core_ids=list(range(NCORES)), **run_kwargs)
    full = np.empty((B, S, D), np.float32)
    for c in range(NCORES):
        b, qo = divmod(c, QSPLIT)
        full[b, qo * SQ_FULL:(qo + 1) * SQ_FULL] = res.results[c]["out"]
    if run_kwargs:
        return full, res
    return full


# revision 16
# speedup vs baseline: 1.4476x; 1.0402x over previous
"""Trainium2 Bass kernel for nn_MultiHeadAttention (B=2, S=4096, D=512, H=8).

Computes: q/k/v = relu(x@W+b) per head, softmax(q k^T / sqrt(64)) v,
out = relu(concat_heads @ Wo + bo).

Sharding: 8 cores = 2 (batch) x 4 (query-slice).  Each core computes full
K/V projections for its batch (redundant across the 4 q-slice cores) and
attention + output projection for its 1024-row query slice.  No collectives;
the host concatenates the 8 output slices.

v2 structure (vs the v1 baseline): attention is organized in "groups" of
(head, ktile-pair): the two ktiles' scores go into one PSUM tile, one exp op
produces fp8 probabilities pT[128, 2, 512], and a single fp8 DoubleRow
matmul (contracting 256 sequence positions) accumulates U.  V is stored in
fp8 with a ones column so U row 64 is the softmax denominator.  exp work is
split between the Scalar engine (exact exp, fp8 out) and the Vector engine
(Schraudolph bit-trick: y=round(s*A+B) as int8, bits reinterpreted as fp8 —
~2.7% rms which the softmax normalization tolerates) so the two engines run
concurrently.  Denominator reciprocals run on ACT as exp(-ln(d)) over the
whole [1, 1024] row; normalization multiplies run on gpsimd.
"""

import numpy as np
import ml_dtypes

import concourse.bass as bass
import concourse.mybir as mybir
import concourse.tile as tile
from concourse import bacc
from concourse import bass_utils
from concourse import hw_specs


def _patch_act_tables():
    """Make exp/relu/ln all resolve to the one table set that contains all
    three (natural_log_exp_and_others).  The load-insertion pass assigns
    each ACTIVATE the *first* set containing its function, so a kernel
    mixing exp and ln otherwise reloads tables around every ln (~2.7us per
    switch).  Only set *selection* changes; set contents seen by the
    runtime are untouched."""
    if getattr(hw_specs, "_mha_act_patch", False):
        return
    orig = hw_specs.get_activation_tables
    HOME = "natural_log_exp_and_others"
    AF_ = mybir.ActivationFunctionType

    def patched(arch):
        tables = orig(arch)
        if HOME not in tables:
            return tables
        out = {}
        for name, funcs in tables.items():
            if name != HOME:
                funcs = funcs - {AF_.Exp, AF_.Relu, AF_.Ln}
            out[name] = funcs
        return out

    hw_specs.get_activation_tables = patched
    bacc.get_activation_tables = patched
    hw_specs._mha_act_patch = True

F32 = mybir.dt.float32
BF16 = mybir.dt.bfloat16
FP8 = mybir.dt.float8e4
I8 = mybir.dt.int8
AF = mybir.ActivationFunctionType
ALU = mybir.AluOpType
DR = mybir.MatmulPerfMode.DoubleRow

P = 128
D = 512
H = 8
DH = 64
DT = D // P  # 4 (also = number of head pairs)
B = 2
S = 4096
NCORES = 8
QSPLIT = 4
SQ_FULL = S // QSPLIT  # 1024 query rows per core
QC = 512               # q-chunk (matmul free dim / PSUM bank width)
VP = 80                # padded V row stride (65 used; 80 keeps fp8 16B align)

# exp folding: pT = exp(s/8 + EXPB); the e^EXPB factor cancels in normalize.
EXPB = -2.9
LOG2E = 1.4426950408889634
# DVE bit-trick: int8(round(s*A8 + B8)) bits == fp8e4(exp(s/8 + EXPB))
A8 = (1 << 3) * LOG2E / 8.0
C8 = 0.35
B8 = 7 * (1 << 3) + (1 << 3) * LOG2E * EXPB - C8

# exp engine schedule, indexed by ktile ('a'=ACT exact, 'd'=DVE bit-trick)
EXP_PATTERN = "adaadaad"


def build_mha(sk=S, sq=SQ_FULL, skip_vbias=False):
    """Build the SPMD Bass program (identical on all cores).

    All inputs arrive pre-tiled by the host into exact SBUF layout
    ([128 partitions, contiguous free bytes]) so every load is a max-packet
    linear DMA."""
    _patch_act_tables()
    nc = bacc.Bacc("TRN2", target_bir_lowering=False, debug=False,
                   num_devices=NCORES)

    xT_d = nc.dram_tensor("xT_f8", (P, DT * sk), FP8,
                          kind="ExternalInput").ap()  # chunk-major, see prep
    xqT_d = nc.dram_tensor("xqT_f8", (P, DT * sq), FP8,
                           kind="ExternalInput").ap()
    w_dram = {}
    for n in ("wq", "wk", "wv"):
        w_dram[n] = nc.dram_tensor(n, (P, DT * D), FP8,
                                   kind="ExternalInput").ap()
    w_dram["wo"] = nc.dram_tensor("wo", (P, DT * D), BF16,
                                  kind="ExternalInput").ap()
    b_dram = {
        "bq": nc.dram_tensor("bq", (P, DT), F32, kind="ExternalInput").ap(),
        "bk": nc.dram_tensor("bk", (P, DT), F32, kind="ExternalInput").ap(),
        "bv": nc.dram_tensor("bv", (1, D), BF16, kind="ExternalInput").ap(),
        "bo": nc.dram_tensor("bo", (1, D), BF16, kind="ExternalInput").ap(),
    }
    out = nc.dram_tensor("out", (sq, D), F32, kind="ExternalOutput").ap()

    with tile.TileContext(nc) as tc:
        _build_tile(tc, xT_d, xqT_d, w_dram, b_dram, out, sk, sq,
                    skip_vbias)

    nc.compile()
    return nc


def _build_tile(tc, xT_d, xqT_d, w_dram, b_dram, out, sk, sq,
                skip_vbias=False):
    nc = tc.nc
    SK_T = sk // P            # ktiles of the key/value sequence (32)
    NKTP = SK_T // 2          # ktile pairs per head (16)
    SQ_T = sq // P
    NQC = sq // QC            # q chunks per core (2)
    CH = min(4, SK_T)         # stiles per projection chunk
    NCH = SK_T // CH

    with (
        tc.tile_pool(name="singles", bufs=1) as singles,
        tc.tile_pool(name="work", bufs=3) as work,
        tc.tile_pool(name="psum", bufs=2, space="PSUM") as psum,
    ):
        # ---- startup: only what Q-proj pair 0 needs, first ----
        w_bf = {}
        w_bf["wq"] = singles.tile([P, DT, D], FP8, name="wq_f8")
        nc.sync.dma_start(w_bf["wq"], w_dram["wq"].rearrange(
            "p (t n) -> p t n", t=DT))
        b_col = {}
        b_col["bq"] = singles.tile([P, DT], F32, name="bq_col")
        nc.sync.dma_start(b_col["bq"], b_dram["bq"])
        xTq = singles.tile([P, DT, sq], FP8)
        nc.sync.dma_start(xTq, xqT_d.rearrange("p (t s) -> p t s", t=DT))

        QT = singles.tile([P, DT, sq], BF16)

        def qproj(j, nq):
            psQ = psum.tile([P, QC], F32, tag="proj", name="psQ")
            for t2 in range(DT // 2):
                nc.tensor.matmul(
                    psQ, w_bf["wq"][:, 2 * t2:2 * t2 + 2, j * P:(j + 1) * P],
                    xTq[:, 2 * t2:2 * t2 + 2, nq * QC:(nq + 1) * QC],
                    start=(t2 == 0), stop=(t2 == DT // 2 - 1),
                    perf_mode=DR)
            nc.vector.tensor_scalar(
                QT[:, j, nq * QC:(nq + 1) * QC], psQ,
                b_col["bq"][:, j:j + 1], 0.0, op0=ALU.add, op1=ALU.max)

        qproj(0, 0)
        if NQC > 1:
            qproj(0, 1)

        # ---- K-proj deps next (attention can start before V exists) ----
        b_row = {}
        w_bf["wk"] = singles.tile([P, DT, D], FP8, name="wk_f8")
        nc.sync.dma_start(w_bf["wk"], w_dram["wk"].rearrange(
            "p (t n) -> p t n", t=DT))
        b_col["bk"] = singles.tile([P, DT], F32, name="bk_col")
        nc.sync.dma_start(b_col["bk"], b_dram["bk"])
        CHP = CH * P
        xT = singles.tile([P, NCH, DT, CHP], FP8)
        xT_src = xT_d.rearrange("p (n t s) -> p n t s", n=NCH, t=DT)
        nc.sync.dma_start(xT[:, 0], xT_src[:, 0])
        for n in ("wv", "wo"):
            dt_n = BF16 if n == "wo" else FP8
            wb = singles.tile([P, DT, D], dt_n, name=f"{n}_w")
            nc.sync.dma_start(wb, w_dram[n].rearrange(
                "p (t n) -> p t n", t=DT))
            w_bf[n] = wb
            if n == "wv" and not skip_vbias:
                br = singles.tile([1, D], BF16, name="bv_row")
                nc.sync.dma_start(br, b_dram["bv"])
                b_row["bv"] = br
        br = singles.tile([1, D], BF16, name="bo_row")
        nc.sync.dma_start(br, b_dram["bo"])
        b_row["bo"] = br

        # ---- persistent SBUF tensors ----
        bias_t = singles.tile([P, 1], F32)
        nc.vector.memset(bias_t, EXPB)
        xT1 = None
        if not skip_vbias:
            xT1 = singles.tile([1, sk], BF16)
            nc.vector.memset(xT1, 1.0)
        KT = singles.tile([P, DT, sk], BF16)
        V_pad = singles.tile([P, NKTP, H, 2, VP], FP8)
        nc.vector.memset(V_pad[:, :, :, :, DH:DH + 1], 1.0)
        OT = singles.tile([P, DT, sq], BF16)
        OT1 = singles.tile([1, sq], BF16)
        nc.vector.memset(OT1, 1.0)

        # PSUM tags: "proj" 2x1 banks, "scores" 2x2 banks, "psU" 1x2 = 8
        def vproj(st):
            n, si = st // CH, st % CH
            psV = psum.tile([P, D], F32, tag="proj", name="psV")
            for t2 in range(DT // 2):
                nc.tensor.matmul(
                    psV, xT[:, n, 2 * t2:2 * t2 + 2, si * P:(si + 1) * P],
                    w_bf["wv"][:, 2 * t2:2 * t2 + 2, :],
                    start=(t2 == 0),
                    stop=(skip_vbias and t2 == DT // 2 - 1),
                    perf_mode=DR)
            if not skip_vbias:
                nc.tensor.matmul(psV, xT1[:, st * P:(st + 1) * P],
                                 b_row["bv"], start=False, stop=True)
            nc.vector.tensor_scalar_max(
                V_pad[:, st // 2, :, st % 2, 0:DH],
                psV.rearrange("p (h d) -> p h d", h=H), 0.0)

        def kproj(j, n):
            psK = psum.tile([P, CH * P], F32, tag="proj", name="psK")
            for t2 in range(DT // 2):
                nc.tensor.matmul(
                    psK, w_bf["wk"][:, 2 * t2:2 * t2 + 2, j * P:(j + 1) * P],
                    xT[:, n, 2 * t2:2 * t2 + 2, :],
                    start=(t2 == 0), stop=(t2 == DT // 2 - 1),
                    perf_mode=DR)
            nc.vector.tensor_scalar(
                KT[:, j, n * CH * P:(n + 1) * CH * P], psK,
                b_col["bk"][:, j:j + 1], 0.0, op0=ALU.add, op1=ALU.max)

        # fp8 probability ring: slot kt%RING holds exp'd scores for both
        # heads of one ktile; the U matmul reads two adjacent slots with a
        # strided DoubleRow access pattern.
        RING = 8
        PT = singles.tile([P, RING, 2, QC], FP8, name="PT_ring")

        def qk1(j, qc, kt, eng):
            """Scores + exp for BOTH heads of pair j at ktile kt.  The two
            QK matmuls sit in different PE row groups (partitions 0-63 vs
            64-127) and run concurrently; one 1024-wide exp op (eng 'a' =
            ACT exact exp->fp8, 'd' = DVE bit-trick int8-as-fp8) covers
            both heads."""
            q0 = qc * QC
            psS = psum.tile([P, 2, QC], F32, tag="scores", bufs=2,
                            name="psS")
            for a in (0, 1):
                h0 = a * DH
                nc.tensor.matmul(
                    psS[:, a, :],
                    KT[h0:h0 + DH, j, kt * P:(kt + 1) * P],
                    QT[h0:h0 + DH, j, q0:q0 + QC], start=True, stop=True)
            slot = kt % RING
            pT_f = PT[:, slot].rearrange("p a b -> p (a b)")
            psS_f = psS.rearrange("p a b -> p (a b)")
            if eng == "a":
                nc.scalar.activation(pT_f, psS_f, AF.Exp, scale=0.125,
                                     bias=bias_t)
            else:
                nc.vector.tensor_scalar(pT_f.bitcast(I8), psS_f, A8, B8,
                                        op0=ALU.mult, op1=ALU.add)

        def u_pair(j, tp, psU):
            """DoubleRow U matmuls for both heads of ktile pair tp, reading
            ring slots (2tp)%RING, (2tp)%RING+1 (slot stride 2*QC fp8)."""
            s0 = (2 * tp) % RING
            for a in (0, 1):
                nc.tensor.matmul(
                    psU[:, a, :], V_pad[:, tp, 2 * j + a, :, 0:DH + 1],
                    PT[:, s0:s0 + 2, a, :],
                    start=(tp == 0), stop=(tp == NKTP - 1), perf_mode=DR)

        brc_sink = {}

        def finish_block(j, qc, psU):
            """U done for both heads: copy U rows out of PSUM, compute
            1/denominator on ACT (exp(-ln d)), then normalize on gpsimd."""
            q0 = qc * QC
            ucs = work.tile([DH, 2, QC], F32, tag="ucopy", bufs=2,
                            name="ucs")
            nc.vector.tensor_copy(ucs, psU[0:DH])
            # Ln reads the denominator row at partition 64 and lands it at
            # partition 0 (ACT maps partitions relative to the AP base)
            lnd = work.tile([1, 2 * QC], F32, tag="lnd", bufs=2, name="lnd")
            nc.scalar.activation(
                lnd, psU[DH:DH + 1].rearrange("p a b -> p (a b)"), AF.Ln)
            rcp = work.tile([1, 2 * QC], F32, tag="rcp", bufs=2, name="rcp")
            nc.scalar.activation(rcp, lnd, AF.Exp, scale=-1.0)
            for a in (0, 1):
                h0 = a * DH
                brc = work.tile([DH, QC], F32, tag="brc", bufs=4,
                                name="brc")
                nc.gpsimd.partition_broadcast(
                    brc, rcp[0:1, a * QC:a * QC + QC])
                if a == 0:
                    # all partition bases align (0) -> safe on gpsimd
                    nc.gpsimd.tensor_tensor(
                        OT[0:DH, j, q0:q0 + QC], ucs[:, 0, :], brc,
                        op=ALU.mult)
                else:
                    # dest base 64: DVE handles relative partition bases
                    nc.vector.tensor_mul(
                        OT[h0:h0 + DH, j, q0:q0 + QC], ucs[:, a, :], brc)
                brc_sink[(j, qc)] = brc

        def attn_span(j, qc, kts, psU, fillers=()):
            """Emit one attention block: per ktile a QK pair + exp
            (engines alternating by ktile), with the U matmul pair lagging
            two ktiles behind so the in-order PE never waits on an exp.
            Fillers (deferred projections) slot in between ktiles."""
            fillers = list(fillers)
            spacing = max(1, len(kts) // (len(fillers) + 1))
            for i, kt in enumerate(kts):
                eng = EXP_PATTERN[kt % len(EXP_PATTERN)]
                qk1(j, qc, kt, eng)
                if kt >= 3 and kt % 2 == 1:
                    u_pair(j, (kt - 3) // 2, psU)
                if fillers and (i + 1) % spacing == 0:
                    fillers.pop(0)()
            for f in fillers:
                f()
            if kts[-1] == SK_T - 1:
                u_pair(j, NKTP - 1, psU)
                finish_block(j, qc, psU)

        def new_psU():
            return psum.tile([DH + 1, 2, QC], F32, tag="psU", bufs=1,
                             name="psU")

        def outproj(qt):
            # bias matmul first: it reads OT1, whose re-write after the last
            # normalize acts as a scheduling gate for the whole chain
            psO = psum.tile([P, D], F32, tag="proj", name="psO")
            nc.tensor.matmul(psO, OT1[:, qt * P:(qt + 1) * P],
                             b_row["bo"], start=True, stop=False)
            for j in range(DT):
                nc.tensor.matmul(psO, OT[:, j, qt * P:(qt + 1) * P],
                                 w_bf["wo"][:, j, :],
                                 start=False, stop=(j == DT - 1))
            o_sb = work.tile([P, D], F32, tag="osb", bufs=2, name="o_sb")
            nc.scalar.activation(o_sb, psO, AF.Relu)
            nc.sync.dma_start(out[qt * P:(qt + 1) * P, :], o_sb)

        def gate_outproj(blk):
            """No-op rewrite of OT1 (max(1, rcp<1) == 1) that depends on
            block `blk`'s normalize chain — gates the outproj chains (which
            start with an OT1-reading bias matmul) behind it."""
            brc = brc_sink[blk]
            nc.vector.tensor_scalar(OT1, OT1, brc[0:1, 0:1], None,
                                    op0=ALU.max)

        # ---- chunk loop: x load + V proj + K proj(pair 0) + attn(0, 0);
        # exps overlap the vproj matmuls, U runs after its V is written ----
        psU0 = new_psU()
        for n in range(NCH):
            if n > 0:
                nc.sync.dma_start(xT[:, n], xT_src[:, n])
            kproj(0, n)
            for i in range(CH // 2):
                kt0 = n * CH + 2 * i
                qk1(0, 0, kt0, EXP_PATTERN[kt0 % len(EXP_PATTERN)])
                qk1(0, 0, kt0 + 1,
                    EXP_PATTERN[(kt0 + 1) % len(EXP_PATTERN)])
                vproj(kt0)
                vproj(kt0 + 1)
            for i in range(CH // 2):
                u_pair(0, n * CH // 2 + i, psU0)
            if (n + 1) * CH == SK_T:
                finish_block(0, 0, psU0)

        # ---- remaining blocks, qc-major; fillers carry the next block's
        # projections plus the first-half output projections ----
        blocks = [(j, 0) for j in range(1, DT)]
        blocks += [(j, 1) for j in range(DT)] if NQC > 1 else []
        owed = {blk: [] for blk in blocks}
        for (j, qc) in blocks:
            if not (j == 0 and qc <= 1):
                owed[(j, qc)].append(lambda j=j, qc=qc: qproj(j, qc))
            if qc == 0 and j >= 1:
                for n in range(NCH):
                    owed[(j, qc)].append(lambda j=j, n=n: kproj(j, n))
        # first-half outproj: OT rows for qc=0 complete after block (DT-1, 0);
        # run them inside the following blocks
        if NQC > 1:
            mid_i = blocks.index((0, 1))
            later = blocks[mid_i + 1]
            owed[later].append(lambda: gate_outproj((DT - 1, 0)))
            for qt in range(SQ_T // NQC):
                owed[later].append(lambda qt=qt: outproj(qt))

        for f in owed[blocks[0]]:
            f()
        for bi, (j, qc) in enumerate(blocks):
            fillers = []
            if bi + 1 < len(blocks):
                fillers += owed[blocks[bi + 1]]
            psU = new_psU()
            attn_span(j, qc, list(range(SK_T)), psU, fillers)

        # ---- tail: last block's normalize + remaining output rows ----
        gate_outproj(blocks[-1])
        qt_lo = SQ_T // NQC if NQC > 1 else 0
        for qt in range(qt_lo, SQ_T):
            outproj(qt)


_NC_CACHE = {}


def _get_nc(sk=S, sq=SQ_FULL, skip_vbias=False):
    key = (sk, sq, skip_vbias)
    if key not in _NC_CACHE:
        _NC_CACHE[key] = build_mha(sk, sq, skip_vbias)
    return _NC_CACHE[key]


def _tile_rows(a):
    """[D, n] -> SBUF layout [P, DT*n]: partition p gets rows p, 128+p, ..."""
    Dd, n = a.shape
    t = Dd // P
    return np.ascontiguousarray(
        a.reshape(t, P, n).transpose(1, 0, 2).reshape(P, t * n))


def _tile_chunks(a, chp):
    """[D, sk] -> chunk-major SBUF layout [P, NCH*DT*chp]: per partition,
    sequence chunks outermost so each chunk is one contiguous linear DMA."""
    Dd, sk = a.shape
    t, nch = Dd // P, sk // chp
    return np.ascontiguousarray(
        a.reshape(t, P, nch, chp).transpose(1, 2, 0, 3).reshape(P, -1))


def prep_inputs(x, Wq, bq, Wk, bk, Wv, bv, Wo, bo):
    """Host-side sharding/layout prep: bf16 casts, feature-major transpose,
    SBUF pre-tiling.  Returns the 8 per-core input maps."""
    bf = ml_dtypes.bfloat16
    f8 = ml_dtypes.float8_e4m3
    x = np.asarray(x, dtype=np.float32)
    shared = {
        "wq": _tile_rows(np.asarray(Wq, np.float32).astype(f8)),
        "wk": _tile_rows(np.asarray(Wk, np.float32).astype(f8)),
        "wv": _tile_rows(np.asarray(Wv, np.float32).astype(f8)),
        "wo": _tile_rows(np.asarray(Wo, np.float32).astype(bf)),
        "bq": np.ascontiguousarray(
            np.asarray(bq, np.float32).reshape(DT, P).T),
        "bk": np.ascontiguousarray(
            np.asarray(bk, np.float32).reshape(DT, P).T),
        "bv": np.asarray(bv, np.float32).astype(bf).reshape(1, D),
        "bo": np.asarray(bo, np.float32).astype(bf).reshape(1, D),
    }
    xT_b = [x[b].T.astype(f8) for b in range(B)]
    xT_tiled = [_tile_chunks(xb, 4 * P) for xb in xT_b]
    in_maps = []
    for c in range(NCORES):
        b, qo = divmod(c, QSPLIT)
        m = dict(shared)
        m["xT_f8"] = xT_tiled[b]
        m["xqT_f8"] = _tile_rows(
            xT_b[b][:, qo * SQ_FULL:(qo + 1) * SQ_FULL])
        in_maps.append(m)
    return in_maps


def kernel(x, Wq, bq, Wk, bk, Wv, bv, Wo, bo, **run_kwargs):
    """Full-input entry point: shards across 8 NeuronCores, returns full out."""
    in_maps = prep_inputs(x, Wq, bq, Wk, bk, Wv, bv, Wo, bo)
    nc = _get_nc(skip_vbias=bool(np.all(np.asarray(bv) == 0)))
    res = bass_utils.run_bass_kernel_spmd(
        nc, in_maps, core_ids=list(range(NCORES)), **run_kwargs)
    full = np.empty((B, S, D), np.float32)
    for c in range(NCORES):
        b, qo = divmod(c, QSPLIT)
        full[b, qo * SQ_FULL:(qo + 1) * SQ_FULL] = res.results[c]["out"]
    if run_kwargs:
        return full, res
    return full
